# revision 2
# baseline (speedup 1.0000x reference)
"""Trainium2 Bass kernel v2 for nn_DataONEEncoder (2-layer GRU + LN + pool + proj + GELU).

Data-parallel over batch: B=256 -> 32 per core on 8 NeuronCores, no collectives.

Key structure (vs v1): the two GRU layer scans are software-pipelined on each
core (layer-1 scan runs 2 chunks = 32 steps behind layer 0), with the input
GEMMs (gx0/gx1), the LN/pool epilogue, and all DMA spread as background work
items between scan steps. All intermediate streams (gx chunks, h chunks) stay
SBUF-resident in bf16; pre-activations for r/z and the b_hh(n) bias are
preloaded into PSUM via cheap identity matmuls so the gate matmuls accumulate
on top; the z gate's weights are negated on the host so sigmoid directly
yields (1-z) and r/z share one activation instruction.
"""

import os
import numpy as np
import ml_dtypes

import concourse.bass as bass
from concourse import bacc
import concourse.mybir as mybir
import concourse.tile as tile
from concourse.alu_op_type import AluOpType
from concourse.bass import ts, ds

B, T, F, H = 256, 512, 65, 512
NCORES = 8
BL = B // NCORES          # 32 batch per core
H3 = 3 * H                # 1536
NJ = H3 // 128            # 12 gate tiles
NK = H // 128             # 4 hidden tiles
CS = 16                   # scan steps per chunk
CT = CS * BL              # tokens per chunk (512)
NCH = T // CS             # 32 chunks
LAG = 2 * CS              # scan1 runs this many steps behind scan0
EPS = 1e-5

f32 = mybir.dt.float32
f32r = mybir.dt.float32r
bf16 = mybir.dt.bfloat16
AF = mybir.ActivationFunctionType

SIM_MODE = os.environ.get("KERNEL_SIM", "0") == "1"   # CoreSim lacks Gelu
# carried h state: f32 (extra cast op) vs bf16 (one less op, more rounding)
HF32 = os.environ.get("KERNEL_HF32", "0") == "1"
MAXBG = int(os.environ.get("KERNEL_MAXBG", "3"))
# 0: DVE adds for gx_rz/bhn; 1: bulk-region PSUM preload (BROKEN — cross-region
# accumulation groups miscompute); 2: per-j-tile identity preload
PRELOAD = int(os.environ.get("KERNEL_PRELOAD", "2"))
DEBUG = os.environ.get("KERNEL_DEBUG", "0") == "1"


def r32(ap):
    return ap.bitcast(f32r)


def build_nc():
    nc = bacc.Bacc()

    # ---- external inputs (host pre-laid-out, see prep_* below) ----
    xmT = nc.declare_dram_parameter("xmT", [2 * F, T * BL], f32r, isOutput=False)
    w0T = nc.declare_dram_parameter("w0T", [F, 2, H3], f32r, isOutput=False)
    w1T = nc.declare_dram_parameter("w1T", [128, NK, H3], bf16, isOutput=False)
    whh0 = nc.declare_dram_parameter("whh0", [128, NJ, NK, 128], bf16, isOutput=False)
    whh1 = nc.declare_dram_parameter("whh1", [128, NJ, NK, 128], bf16, isOutput=False)
    gb0 = nc.declare_dram_parameter("gb0", [128, NJ], f32, isOutput=False)
    gb1 = nc.declare_dram_parameter("gb1", [128, NJ], f32, isOutput=False)
    bhn0 = nc.declare_dram_parameter("bhn0", [128, NK], f32, isOutput=False)
    bhn1 = nc.declare_dram_parameter("bhn1", [128, NK], f32, isOutput=False)
    lng = nc.declare_dram_parameter("lng", [128, NK], f32, isOutput=False)
    lnb2 = nc.declare_dram_parameter("lnb2", [128, NK], f32, isOutput=False)  # 2*ln_b
    wpT = nc.declare_dram_parameter("wpT", [128, NK, 256], f32r, isOutput=False)
    bp = nc.declare_dram_parameter("bp", [128, 2], f32, isOutput=False)
    ident = nc.declare_dram_parameter("ident", [128, 128], bf16, isOutput=False)
    out = nc.declare_dram_parameter("out", [2, 128, BL], f32, isOutput=True)
    if DEBUG:
        dbg = nc.declare_dram_parameter("dbg", [3, 128, NJ, CT], bf16, isOutput=True)

    with tile.TileContext(nc) as tc:
        with tc.tile_pool(name="consts", bufs=1) as consts, \
             tc.tile_pool(name="xmp", bufs=2) as xmp, \
             tc.tile_pool(name="gx0p", bufs=2) as gx0p, \
             tc.tile_pool(name="gx1p", bufs=2) as gx1p, \
             tc.tile_pool(name="h1p", bufs=2) as h1p, \
             tc.tile_pool(name="h2p", bufs=2) as h2p, \
             tc.tile_pool(name="hs0p", bufs=2) as hs0p, \
             tc.tile_pool(name="hs1p", bufs=2) as hs1p, \
             tc.tile_pool(name="sct", bufs=2) as sct, \
             tc.tile_pool(name="ep", bufs=2) as ep, \
             tc.tile_pool(name="sqp", bufs=1) as sqp, \
             tc.tile_pool(name="acc", bufs=1) as accp, \
             tc.tile_pool(name="scps", bufs=2, space="PSUM") as scps0, \
             tc.tile_pool(name="scps1", bufs=2, space="PSUM") as scps1, \
             tc.tile_pool(name="gps", bufs=2, space="PSUM") as gps, \
             tc.tile_pool(name="eps", bufs=1, space="PSUM") as epsp:

            # ---- load constants to SBUF ----
            w0_sb = consts.tile([F, 2, H3], f32r)
            nc.sync.dma_start(out=w0_sb, in_=w0T[:])
            w1_sb = consts.tile([128, NK, H3], bf16)
            nc.sync.dma_start(out=w1_sb, in_=w1T[:])
            whh_sb = [consts.tile([128, NJ, NK, 128], bf16, name=f"whh{i}_sb")
                      for i in range(2)]
            nc.sync.dma_start(out=whh_sb[0], in_=whh0[:])
            nc.sync.dma_start(out=whh_sb[1], in_=whh1[:])
            gb_sb = [consts.tile([128, NJ], f32, name=f"gb{i}_sb") for i in range(2)]
            nc.sync.dma_start(out=gb_sb[0], in_=gb0[:])
            nc.sync.dma_start(out=gb_sb[1], in_=gb1[:])
            # broadcast b_hh(n) over batch -> [128, NK, BL] bf16 (for PSUM preload)
            bhn_small = [consts.tile([128, NK], f32, name=f"bhn{i}_sm") for i in range(2)]
            bhn_sb = [consts.tile([128, NK, BL], bf16, name=f"bhn{i}_sb") for i in range(2)]
            for i, srcp in enumerate((bhn0, bhn1)):
                nc.sync.dma_start(out=bhn_small[i], in_=srcp[:])
                nc.vector.tensor_copy(out=bhn_sb[i],
                                      in_=bhn_small[i].to_broadcast([128, NK, BL]))
            lng_sb = consts.tile([128, NK], f32)
            nc.sync.dma_start(out=lng_sb, in_=lng[:])
            lnb2_sb = consts.tile([128, NK], f32)
            nc.sync.dma_start(out=lnb2_sb, in_=lnb2[:])
            wp_sb = consts.tile([128, NK, 256], f32r)
            nc.sync.dma_start(out=wp_sb, in_=wpT[:])
            bp_sb = consts.tile([128, 2], f32)
            nc.sync.dma_start(out=bp_sb, in_=bp[:])
            id_sb = consts.tile([128, 128], bf16)
            nc.sync.dma_start(out=id_sb, in_=ident[:])
            ones_stage = consts.tile([128, 128], f32)
            nc.vector.memset(ones_stage, 1.0)
            ones_colb = consts.tile([128, 1], bf16)   # lhsT for partition-sum (bf16)
            nc.vector.tensor_copy(out=ones_colb, in_=ones_stage[:, 0:1])
            ones_row = consts.tile([1, 128], f32r)    # lhsT for partition-broadcast
            nc.vector.tensor_copy(out=ones_row, in_=ones_stage[0:1, :])
            eps_sb = consts.tile([1, 1], f32)
            nc.vector.memset(eps_sb, EPS)
            zh_b = consts.tile([128, NK, BL], bf16)   # zero initial h (mm input)
            nc.vector.memset(zh_b, 0.0)
            zh_f = consts.tile([128, NK, BL], f32)    # zero initial h (gate input)
            nc.vector.memset(zh_f, 0.0)
            tc.strict_bb_all_engine_barrier()

            # ---------------- background work-item machinery ----------------
            bgq = []

            def pop_bg(n):
                for _ in range(n):
                    if bgq:
                        bgq.pop(0)()

            # chunk tile registries (python-side handles)
            xm_t = {}
            gx0_t = {}
            gx1_t = {}
            h1_t = {}
            h2_t = {}

            def xm_dma(c):
                t_ = xmp.tile([F, 2, CT], f32r, tag="xm")
                xm_t[c] = t_
                nc.sync.dma_start(
                    out=t_, in_=xmT[:, ds(c * CT, CT)].rearrange(
                        "(k f) t -> f k t", k=2))

            def drain(dst, ps, gb_col, j):
                """PSUM -> bf16 chunk drain + bias on Act (DVE is chain-critical,
                keep it clean)."""
                nc.scalar.activation(out=dst, in_=ps, func=AF.Identity, bias=gb_col)

            def a_item(c, j):
                """gx0 chunk c, gate tile j: 2 matmuls + drain."""
                if j == 0:
                    gx0_t[c] = gx0p.tile([128, NJ, CT], bf16, tag="gx0", name=f"gx0c{c}")
                xm_sb = xm_t[c]
                ps = gps.tile([128, CT], f32, tag="g")
                nc.tensor.matmul(ps, r32(w0_sb[:, 0, ts(j, 128)]), r32(xm_sb[:, 0, :]),
                                 start=True, stop=False)
                nc.tensor.matmul(ps, r32(w0_sb[:, 1, ts(j, 128)]), r32(xm_sb[:, 1, :]),
                                 start=False, stop=True)
                drain(gx0_t[c][:, j, :], ps, gb_sb[0][:, j:j + 1], j)
                if DEBUG and c == 0 and j == NJ - 1:
                    nc.sync.dma_start(out=dbg[0], in_=gx0_t[c])
                if j == NJ - 1:
                    xm_t.pop(c)

            def c_item(c, j):
                """gx1 chunk c, gate tile j: 4 matmuls + act drain."""
                if j == 0:
                    gx1_t[c] = gx1p.tile([128, NJ, CT], bf16, tag="gx1", name=f"gx1c{c}")
                h1c = h1_t[c]
                ps = gps.tile([128, CT], f32, tag="g")
                for k in range(NK):
                    nc.tensor.matmul(ps, w1_sb[:, k, ts(j, 128)], h1c[:, k, :],
                                     start=(k == 0), stop=(k == NK - 1))
                drain(gx1_t[c][:, j, :], ps, gb_sb[1][:, j:j + 1], j + 1)
                if j == NJ - 1:
                    h1_t.pop(c)

            # ---------------- scan step ----------------
            # per-layer python state
            prev_hb = [zh_b, zh_b]   # bf16 h_{t-1} (matmul input)
            prev_hf = [zh_f, zh_f]   # f32 h_{t-1} (gate input); unused if not HF32
            scan_cfg = [
                dict(whh=whh_sb[0], bhn=bhn_sb[0], psp=scps0, hp=hs0p, ht=h1_t,
                     hpool=h1p, tag="s0"),
                dict(whh=whh_sb[1], bhn=bhn_sb[1], psp=scps1, hp=hs1p, ht=h2_t,
                     hpool=h2p, tag="s1"),
            ]

            def scan_step(layer, t):
                cfg = scan_cfg[layer]
                c, s = t // CS, t % CS
                sl = ds(s * BL, BL)
                gxc = (gx0_t if layer == 0 else gx1_t)[c]
                if s == 0:
                    cfg["ht"][c] = cfg["hpool"].tile([128, NK, CT], bf16,
                                                     tag=cfg["tag"] + "h",
                                                     name=cfg["tag"] + f"hc{c}")
                hc = cfg["ht"][c]
                hb = prev_hb[layer]
                hf = prev_hf[layer] if HF32 else hb
                whh, bhn = cfg["whh"], cfg["bhn"]
                tg = cfg["tag"]

                ps = cfg["psp"].tile([128, NJ, BL], f32, tag=tg + "ps")
                if PRELOAD == 1:
                    # preload: i_r|i_z into r/z region, b_hh(n) into n region
                    nc.tensor.matmul(ps[:, 0:8, :], id_sb, gxc[:, 0:8, sl],
                                     start=True, stop=False)
                    nc.tensor.matmul(ps[:, 8:12, :], id_sb, bhn,
                                     start=True, stop=False)
                # issue order r (0-3), n (8-11), z (4-7): r-sigmoid overlaps n/z
                # matmuls; the n-gate chain (the long pole) starts earliest.
                for j in (0, 1, 2, 3, 8, 9, 10, 11, 4, 5, 6, 7):
                    if PRELOAD == 2:
                        src = gxc[:, j, sl] if j < 8 else bhn[:, j - 8, :]
                        nc.tensor.matmul(ps[:, j, :], id_sb, src,
                                         start=True, stop=False)
                    for k in range(NK):
                        nc.tensor.matmul(ps[:, j, :], whh[:, j, k, :], hb[:, k, :],
                                         start=(PRELOAD == 0 and k == 0),
                                         stop=(k == NK - 1))
                r = sct.tile([128, 4, BL], bf16, tag=tg + "r")
                zc = sct.tile([128, 4, BL], bf16, tag=tg + "zc")
                nh = sct.tile([128, NK, BL], f32, tag=tg + "nh")
                if PRELOAD:
                    nc.scalar.activation(out=r, in_=ps[:, 0:4, :], func=AF.Sigmoid)
                    nc.vector.tensor_mul(nh, ps[:, 8:12, :], r)
                else:
                    rzpre = sct.tile([128, 8, BL], f32, tag=tg + "rzpre")
                    nc.vector.tensor_add(rzpre, ps[:, 0:8, :], gxc[:, 0:8, sl])
                    nc.scalar.activation(out=r, in_=rzpre[:, 0:4, :], func=AF.Sigmoid)
                    nb = sct.tile([128, NK, BL], f32, tag=tg + "nb")
                    nc.vector.tensor_add(nb, ps[:, 8:12, :], bhn)
                    nc.vector.tensor_mul(nh, nb, r)
                npre = sct.tile([128, NK, BL], f32, tag=tg + "npre")
                nc.vector.tensor_add(npre, nh, gxc[:, 8:12, sl])
                n = sct.tile([128, NK, BL], f32, tag=tg + "n")
                nc.scalar.activation(out=n, in_=npre, func=AF.Tanh)
                if PRELOAD:
                    nc.scalar.activation(out=zc, in_=ps[:, 4:8, :], func=AF.Sigmoid)
                else:
                    nc.scalar.activation(out=zc, in_=rzpre[:, 4:8, :], func=AF.Sigmoid)
                nd = sct.tile([128, NK, BL], f32, tag=tg + "nd")
                nc.vector.tensor_sub(nd, n, hf)
                znd = sct.tile([128, NK, BL], f32, tag=tg + "znd")
                nc.vector.tensor_mul(znd, zc, nd)
                hout = hc[:, :, sl]
                if HF32:
                    hnf = cfg["hp"].tile([128, NK, BL], f32, tag=tg + "hf")
                    nc.vector.tensor_add(hnf, hf, znd)
                    nc.vector.tensor_copy(out=hout, in_=hnf)
                    prev_hf[layer] = hnf
                else:
                    with nc.allow_low_precision(reason="bf16 carried h state"):
                        nc.vector.tensor_add(hout, hf, znd)
                prev_hb[layer] = hout
                if DEBUG and c == 0 and s == CS - 1:
                    nc.sync.dma_start(out=dbg[1 + layer, :, 0:NK, :], in_=hc)

            # ---------------- LN/pool epilogue items ----------------
            acc1 = accp.tile([128, NK, BL], f32)
            nc.vector.memset(acc1, 0.0)
            acc2 = accp.tile([128, BL], f32)
            nc.vector.memset(acc2, 0.0)
            lastpool = accp.tile([128, NK, BL], f32)   # (h_last-mu)*rs of final step
            estate = {}

            def e_items(c):
                """LN stats + pooled accumulation for h2 chunk c (list of items)."""
                items = []

                def sq_item(khalf):
                    if khalf == 0:
                        estate["sq"] = sqp.tile([128, NK, CT], bf16, tag="sq", name="sqt")
                    hc = h2_t[c]
                    s_ = estate["sq"]
                    k0 = khalf * 2
                    nc.scalar.activation(out=s_[:, k0:k0 + 2, :],
                                         in_=hc[:, k0:k0 + 2, :], func=AF.Square)

                def sums_item():
                    hc = h2_t[c]
                    pss = epsp.tile([1, CT], f32, tag="pss")
                    psq = epsp.tile([1, CT], f32, tag="psq")
                    for k in range(NK):
                        nc.tensor.matmul(pss, ones_colb, hc[:, k, :],
                                         start=(k == 0), stop=(k == NK - 1))
                    for k in range(NK):
                        nc.tensor.matmul(psq, ones_colb, estate["sq"][:, k, :],
                                         start=(k == 0), stop=(k == NK - 1))
                    estate["pss"], estate["psq"] = pss, psq  # noqa

                def stats_item():
                    mu = ep.tile([1, CT], f32r, tag="mu")
                    with nc.allow_low_precision(reason="f32r full-width fp32 bits"):
                        nc.vector.tensor_scalar_mul(mu, estate["pss"], 1.0 / H)
                    mu2 = ep.tile([1, CT], f32, tag="mu2")
                    nc.vector.tensor_mul(mu2, mu.bitcast(f32), mu.bitcast(f32))
                    var = ep.tile([1, CT], f32, tag="var")
                    nc.vector.scalar_tensor_tensor(var, estate["psq"], 1.0 / H, mu2,
                                                   op0=AluOpType.mult,
                                                   op1=AluOpType.subtract)
                    sd = ep.tile([1, CT], f32, tag="sd")
                    nc.scalar.activation(out=sd, in_=var, func=AF.Sqrt, bias=eps_sb)
                    rs = ep.tile([1, CT], f32r, tag="rs")
                    with nc.allow_low_precision(reason="f32r full-width fp32 bits"):
                        nc.vector.reciprocal(rs, sd)
                    murs = ep.tile([1, CT], f32, tag="murs")
                    nc.vector.tensor_mul(murs, mu.bitcast(f32), rs.bitcast(f32))
                    estate["mu"], estate["rs"], estate["murs"] = mu, rs, murs

                def bcast_item():
                    s2 = ep.tile([1, BL], f32r, tag="s2")
                    with nc.allow_low_precision(reason="f32r is full-width fp32 bits"):
                        nc.vector.tensor_reduce(
                            s2, estate["murs"].rearrange("p (s b) -> p b s", b=BL),
                            axis=mybir.AxisListType.X, op=AluOpType.add)
                    brs = epsp.tile([128, CT], f32, tag="pss")
                    nc.tensor.matmul(brs, ones_row, estate["rs"])
                    bs2 = epsp.tile([128, BL], f32, tag="psq")
                    nc.tensor.matmul(bs2, ones_row, s2)
                    nc.vector.tensor_add(acc2, acc2, bs2)
                    estate["brs"] = brs

                def wh_item(k):
                    hc = h2_t[c]
                    wh = ep.tile([128, CT], f32, tag="wh")
                    nc.vector.tensor_mul(wh, hc[:, k, :], estate["brs"])
                    red = ep.tile([128, BL], f32, tag="red")
                    nc.vector.tensor_reduce(
                        red, wh.rearrange("p (s b) -> p b s", b=BL),
                        axis=mybir.AxisListType.X, op=AluOpType.add)
                    nc.vector.tensor_add(acc1[:, k, :], acc1[:, k, :], red)

                def last_item():
                    # (h_T-1 - mu)*rs for the final 32 tokens
                    hc = h2_t[c]
                    sl = ds((CS - 1) * BL, BL)
                    bmu = epsp.tile([128, BL], f32, tag="pss")
                    nc.tensor.matmul(bmu, ones_row, estate["mu"][:, sl])
                    brsl = epsp.tile([128, BL], f32, tag="psq")
                    nc.tensor.matmul(brsl, ones_row, estate["rs"][:, sl])
                    for k in range(NK):
                        cen = ep.tile([128, BL], f32, tag="wh")
                        nc.vector.tensor_sub(cen, hc[:, k, sl], bmu)
                        nc.vector.tensor_mul(lastpool[:, k, :], cen, brsl)

                def done_item():
                    h2_t.pop(c)

                items += [lambda kh=kh: sq_item(kh) for kh in range(2)]
                items += [sums_item, stats_item, bcast_item]
                items += [lambda k=k: wh_item(k) for k in range(NK)]
                if c == NCH - 1:
                    items.append(last_item)
                items.append(done_item)
                return items

            # ---------------- prologue ----------------
            for c in range(2):
                xm_dma(c)
            for j in range(NJ):
                a_item(0, j)
            for j in range(NJ):
                a_item(1, j)

            # ---------------- main pipelined loop ----------------
            TEND = T + LAG
            for t in range(TEND):
                if t % CS == 0:
                    p = t // CS
                    newitems = []
                    if p - 1 >= 0 and p - 1 < NCH:
                        newitems += [lambda c=p - 1, j=j: c_item(c, j)
                                     for j in range(NJ)]
                    if p >= 1 and p + 1 < NCH:
                        newitems += [lambda c=p + 1, j=j: a_item(c, j)
                                     for j in range(NJ)]
                    if p + 2 < NCH:
                        newitems.append(lambda c=p + 2: xm_dma(c))
                    if p - 3 >= 0 and p - 3 < NCH:
                        newitems += e_items(p - 3)
                    bgq.extend(newitems)
                if t < T:
                    scan_step(0, t)
                pop_bg(1)
                if t >= LAG:
                    scan_step(1, t - LAG)
                pop_bg(MAXBG - 1)

            # drain remaining background, then last E chunks
            pop_bg(len(bgq))
            for c in (NCH - 2, NCH - 1):
                if c in h2_t:
                    for it in e_items(c):
                        it()

            # ---------------- final: pooled + proj + gelu ----------------
            po = accp.tile([128, NK, BL], f32r)
            q = accp.tile([128, NK, BL], f32)
            inner = accp.tile([128, NK, BL], f32)
            for k in range(NK):
                nc.vector.tensor_sub(q[:, k, :], acc1[:, k, :], acc2)
                nc.vector.scalar_tensor_tensor(inner[:, k, :], q[:, k, :], 1.0 / T,
                                               lastpool[:, k, :],
                                               op0=AluOpType.mult, op1=AluOpType.add)
                with nc.allow_low_precision(reason="f32r full-width fp32 bits"):
                    nc.vector.tensor_scalar(po[:, k, :], inner[:, k, :],
                                            lng_sb[:, k:k + 1], lnb2_sb[:, k:k + 1],
                                            op0=AluOpType.mult, op1=AluOpType.add)
            for jj in range(2):
                psy = gps.tile([128, BL], f32, tag="g")
                for k in range(NK):
                    nc.tensor.matmul(psy, r32(wp_sb[:, k, ts(jj, 128)]),
                                     r32(po[:, k, :]),
                                     start=(k == 0), stop=(k == NK - 1))
                yj = accp.tile([128, BL], f32, name=f"yj{jj}")
                nc.scalar.activation(out=yj, in_=psy,
                                     func=AF.Identity if SIM_MODE else AF.Gelu,
                                     bias=bp_sb[:, jj:jj + 1])
                nc.sync.dma_start(out=out[jj], in_=yj)
    nc.finalize()
    return nc


# ---------------- host-side input prep ----------------

def prep_shared(W_ih0, W_hh0, b_ih0, b_hh0, W_ih1, W_hh1, b_ih1, b_hh1,
                ln_g, ln_b, W_proj, b_proj):
    def neg_z(gmat):
        # negate the z-gate block (rows H..2H of the 3H gate dim), gmat [3H, D]
        g = gmat.copy()
        g[H:2 * H] *= -1.0
        return g

    def whh_tiles(W_hh):
        # [p, j, k, m] = W_hh^T[128k+p, 128j+m], z gate tiles negated
        w = np.ascontiguousarray(neg_z(W_hh).T).reshape(NK, 128, NJ, 128)
        return np.ascontiguousarray(w.transpose(1, 2, 0, 3)).astype(ml_dtypes.bfloat16)

    def fold_bias(b_ih, b_hh):
        g = b_ih.copy()
        g[:2 * H] += b_hh[:2 * H]
        g[H:2 * H] *= -1.0
        return np.ascontiguousarray(g.reshape(NJ, 128).T)  # [128, NJ]

    shared = {}
    w0 = np.ascontiguousarray(neg_z(W_ih0).T)            # [130, 1536]
    shared["w0T"] = np.ascontiguousarray(w0.reshape(2, F, H3).transpose(1, 0, 2))
    w1 = np.ascontiguousarray(neg_z(W_ih1).T)            # [512, 1536]
    shared["w1T"] = np.ascontiguousarray(
        w1.reshape(NK, 128, H3).transpose(1, 0, 2)).astype(ml_dtypes.bfloat16)
    shared["whh0"] = whh_tiles(W_hh0)
    shared["whh1"] = whh_tiles(W_hh1)
    shared["gb0"] = fold_bias(b_ih0, b_hh0)
    shared["gb1"] = fold_bias(b_ih1, b_hh1)
    shared["bhn0"] = np.ascontiguousarray(b_hh0[2 * H:].reshape(NK, 128).T)
    shared["bhn1"] = np.ascontiguousarray(b_hh1[2 * H:].reshape(NK, 128).T)
    shared["lng"] = np.ascontiguousarray(ln_g.reshape(NK, 128).T)
    shared["lnb2"] = np.ascontiguousarray((2.0 * ln_b).reshape(NK, 128).T)
    shared["wpT"] = np.ascontiguousarray(W_proj.T.reshape(NK, 128, 256).transpose(1, 0, 2))
    shared["bp"] = np.ascontiguousarray(b_proj.reshape(2, 128).T)
    shared["ident"] = np.eye(128, dtype=ml_dtypes.bfloat16)
    shared = {k: np.asarray(v, dtype=(ml_dtypes.bfloat16
                                      if k in ("whh0", "whh1", "w1T", "ident")
                                      else np.float32))
              for k, v in shared.items()}
    return shared


def prep_xmT(x_core, mask_core):
    # xmT[f, t*BL + b] = concat(x, mask)[b, t, f]
    xm = np.concatenate([x_core, mask_core.astype(np.float32)], axis=-1)  # [BL,T,2F]
    return np.ascontiguousarray(xm.transpose(2, 1, 0).reshape(2 * F, T * BL),
                                dtype=np.float32)


_CACHE = {}


def _enable_trace_support():
    """Profiling-only shim (used by test.py, not the graded path)."""
    import sys
    import types
    import concourse.bass_utils as bu
    bu.upload_artifacts = lambda tmpdir: "local://" + tmpdir
    try:
        from antenv.axon_hooks import get_axon_ntff_profile_hook  # noqa: F401
        return
    except ImportError:
        pass
    from trn_agent_boot.trn_boot import _ntff_profile_via_ctypes
    hook = _ntff_profile_via_ctypes("/opt/axon/libaxon_pjrt.so")
    mod = types.ModuleType("antenv.axon_hooks")
    mod.get_axon_ntff_profile_hook = lambda: hook
    mod.set_axon_ntff_profile_hook = lambda h: None
    sys.modules["antenv.axon_hooks"] = mod


def kernel(x, mask, W_ih0, W_hh0, b_ih0, b_hh0, W_ih1, W_hh1, b_ih1, b_hh1,
           ln_g, ln_b, W_proj, b_proj):
    from concourse.bass_utils import run_bass_kernel_spmd

    if "nc" not in _CACHE:
        _CACHE["nc"] = build_nc()
    nc = _CACHE["nc"]

    x = np.asarray(x, np.float32)
    mask = np.asarray(mask)
    shared = prep_shared(np.asarray(W_ih0, np.float32), np.asarray(W_hh0, np.float32),
                         np.asarray(b_ih0, np.float32), np.asarray(b_hh0, np.float32),
                         np.asarray(W_ih1, np.float32), np.asarray(W_hh1, np.float32),
                         np.asarray(b_ih1, np.float32), np.asarray(b_hh1, np.float32),
                         np.asarray(ln_g, np.float32), np.asarray(ln_b, np.float32),
                         np.asarray(W_proj, np.float32), np.asarray(b_proj, np.float32))
    in_maps = []
    for c in range(NCORES):
        m = dict(shared)
        m["xmT"] = prep_xmT(x[c * BL:(c + 1) * BL], mask[c * BL:(c + 1) * BL])
        in_maps.append(m)

    trace = os.environ.get("KERNEL_TRACE", "0") == "1"
    kw = {}
    if trace:
        _enable_trace_support()
        kw["tmpdir"] = os.environ.get("KERNEL_TRACE_DIR") or None
    res = run_bass_kernel_spmd(nc, in_maps, list(range(NCORES)), trace=trace, **kw)
    _CACHE["exec_time_ns"] = res.exec_time_ns
    if res.instructions_and_trace is not None:
        _CACHE["trace_path"] = res.instructions_and_trace[1]
    if DEBUG:
        _CACHE["dbg"] = [res.results[c].get("dbg") for c in range(NCORES)]
    outs = []
    for c in range(NCORES):
        y = res.results[c]["out"]          # [2, 128, BL]
        outs.append(y.reshape(256, BL).T)  # [BL, 256]
    return np.ascontiguousarray(np.concatenate(outs, axis=0), dtype=np.float32)
